# revision 1
# baseline (speedup 1.0000x reference)
"""Trainium2 Bass kernel v3 for nn_BinLoss_7103875908252.

loss = mean_i ||features_i - centers[labels_i]||^2, labels = 32*b0 + b1
from histogram-binning target (31 edges per axis, count of v > edge).

Data-parallel over 8 cores (4096 rows each, [128 part x 32 slots],
row i = p*32 + r). Host passes centers NEGATED in fp16 (Cn = -centers),
so F - C is formed by ADDING everywhere. Every chunk ends as
PSUM = F - C and ACT Square+accum drains PSUM pairs:

  PE path (slots r < X): one-hot matmul gather, fp8 DoubleRow:
    PSUM = sum_k Sel[k,i]*Cn8[k,d] (4 DR MMs) + (I @ F16) = F - G.
    Sel per 4-chunk group: broadcast-transposes -> labT4 fp16 -> one
    TT is_equal vs a fully materialized bin-id table -> fp8.
  DMA path (slots r >= X): single-row indirect DMA gathers Cn16 rows
    into g tiles (plain, no compute_op - validated primitive), then
    PSUM = (I @ F16) + (I @ g) on PE.

Final: ACT accum cells -> DVE reduce -> ones matmul -> [1,1]/core.
"""

import numpy as np

P = 128
R = 32
D = 512
K = 1024
NCORES = 8
N = 32768
SHARD = N // NCORES
X = 8                       # slots on the PE path; 32-X on the DMA path
assert X % 4 == 0 and (R - X) % 4 == 0

EDGE_BITS = [
    0x00000000, 0x3d088889, 0x3d888889, 0x3dccccce, 0x3e088889, 0x3e2aaaab,
    0x3e4cccce, 0x3e6eeef0, 0x3e888889, 0x3e99999a, 0x3eaaaaab, 0x3ebbbbbc,
    0x3eccccce, 0x3edddddf, 0x3eeeeef0, 0x3f000000, 0x3f088889, 0x3f111112,
    0x3f19999a, 0x3f222223, 0x3f2aaaab, 0x3f333334, 0x3f3bbbbc, 0x3f444445,
    0x3f4cccce, 0x3f555556, 0x3f5ddddf, 0x3f666667, 0x3f6eeef0, 0x3f777778,
    0x3f800000,
]
EDGES = [float(np.uint32(b).view(np.float32)) for b in EDGE_BITS]
NE = len(EDGES)

_CACHE = {}


def build_bass():
    import os
    from contextlib import ExitStack

    import concourse.bacc as bacc
    import concourse.tile as tile
    from concourse import bass, mybir

    f32 = mybir.dt.float32
    fp16 = mybir.dt.float16
    fp8 = mybir.dt.float8e4
    i32 = mybir.dt.int32
    DRmode = mybir.MatmulPerfMode.DoubleRow
    EQ = mybir.AluOpType.is_equal

    nc = bacc.Bacc(
        "TRN2", target_bir_lowering=False, debug=False, num_devices=NCORES
    )
    feat = nc.dram_tensor("features", [SHARD, D], f32, kind="ExternalInput").ap()
    targ = nc.dram_tensor("target", [SHARD, 2], f32, kind="ExternalInput").ap()
    centn = nc.dram_tensor("centneg16", [K, D], fp16, kind="ExternalInput").ap()
    edgesd = nc.dram_tensor("edges", [P, NE], f32, kind="ExternalInput").ap()
    ioffd = nc.dram_tensor("iofull", [P, 8, 4, P], fp16, kind="ExternalInput").ap()
    idf32d = nc.dram_tensor("identf", [P, P], f32, kind="ExternalInput").ap()
    idf16d = nc.dram_tensor("identh", [P, P], fp16, kind="ExternalInput").ap()
    onesd = nc.dram_tensor("ones1", [P, 1], f32, kind="ExternalInput").ap()
    out = nc.dram_tensor("out", [1, 1], f32, kind="ExternalOutput").ap()

    DBG = bool(os.environ.get("KV2_DEBUG"))
    if DBG:
        d_labf = nc.dram_tensor("d_labf", [P, R], f32, kind="ExternalOutput").ap()
        d_sel = nc.dram_tensor("d_sel", [P, 8, 4, P], fp8, kind="ExternalOutput").ap()
        d_g = nc.dram_tensor("d_g", [P, 4, D], fp16, kind="ExternalOutput").ap()
        d_ps = nc.dram_tensor("d_ps", [P, 2, D], f32, kind="ExternalOutput").ap()
        d_labi = nc.dram_tensor("d_labi", [P, R - X], i32, kind="ExternalOutput").ap()

    NG_PE = X // 4              # PE-path 4-chunk groups
    NG_DMA = (R - X) // 4       # DMA-path 4-chunk groups
    NPAIR = R // 2              # psum pairs over ALL chunks

    with tile.TileContext(nc) as tc, ExitStack() as ctx:
        const_p = ctx.enter_context(tc.tile_pool(name="const", bufs=1))
        work_p = ctx.enter_context(tc.tile_pool(name="work", bufs=1))
        selp = ctx.enter_context(tc.tile_pool(name="sel", bufs=2))
        difp = ctx.enter_context(tc.tile_pool(name="dif", bufs=5))
        sqp = ctx.enter_context(tc.tile_pool(name="sq", bufs=2))
        psB_p = ctx.enter_context(tc.tile_pool(name="psB", bufs=2, space="PSUM"))
        ps2_p = ctx.enter_context(tc.tile_pool(name="ps2", bufs=2, space="PSUM"))
        psS_p = ctx.enter_context(tc.tile_pool(name="psS", bufs=2, space="PSUM"))

        # ---- constants + small inputs (HWDGE) --------------------------
        ttile = work_p.tile([P, R, 2], f32)
        nc.sync.dma_start(ttile[:], targ.rearrange("(p r) c -> p r c", p=P))
        etile = const_p.tile([P, NE], f32)
        nc.sync.dma_start(etile[:], edgesd[:, :])
        ioff = const_p.tile([P, 8, 4, P], fp16)
        nc.sync.dma_start(ioff[:], ioffd[:, :, :, :])
        idf32 = const_p.tile([P, P], f32)
        nc.sync.dma_start(idf32[:], idf32d[:, :])
        idf16 = const_p.tile([P, P], fp16)
        nc.sync.dma_start(idf16[:], idf16d[:, :])
        ones1 = const_p.tile([P, 1], f32)
        nc.sync.dma_start(ones1[:], onesd[:, :])

        feat_re = feat.rearrange("(p r) d -> p r d", p=P)
        centn_re = centn.rearrange("(c j) d -> j c d", j=P)
        C16n = work_p.tile([P, 8, D], fp16)     # -centers fp16 (k%128, k//128)
        nc.sync.dma_start(C16n[:, 0:4, :], centn_re[:, 0:4, :])
        nc.sync.dma_start(C16n[:, 4:8, :], centn_re[:, 4:8, :])

        # ---- feature loads: plain f32 via HWDGE (gpsimd queue = gathers)
        F32 = work_p.tile([P, R, D], f32)
        for g in range(R // 8):
            nc.sync.dma_start(
                F32[:, 8 * g:8 * g + 8, :], feat_re[:, 8 * g:8 * g + 8, :]
            )

        # ---- ACT table prefetch ----------------------------------------
        dummy = const_p.tile([P, 1], fp16)
        dacc = const_p.tile([P, 1], f32)
        nc.scalar.activation(
            out=dummy[:], in_=idf16[:, 0:1],
            func=mybir.ActivationFunctionType.Square, accum_out=dacc[:],
        )

        # ---- binning (DVE) ---------------------------------------------
        labf = work_p.tile([P, R], f32)
        cmp = work_p.tile([P, 32, NE], f32)
        bins = work_p.tile([P, R, 2], f32)
        labm = work_p.tile([P, R], f32)

        def bin_half(h):
            rs = slice(16 * h, 16 * h + 16)
            tv = ttile[:, rs, :].rearrange("p r c -> p (r c)")
            nc.vector.tensor_tensor(
                out=cmp[:],
                in0=tv.unsqueeze(2).broadcast_to([P, 32, NE]),
                in1=etile[:].unsqueeze(1).broadcast_to([P, 32, NE]),
                op=mybir.AluOpType.is_gt,
            )
            nc.vector.tensor_reduce(
                out=bins[:, rs, :].rearrange("p r c -> p (r c)"),
                in_=cmp[:],
                axis=mybir.AxisListType.X,
                op=mybir.AluOpType.add,
            )
            nc.vector.tensor_scalar(
                out=labm[:, rs], in0=bins[:, rs, 0],
                scalar1=float(32.0), scalar2=None, op0=mybir.AluOpType.mult,
            )
            nc.vector.tensor_tensor(
                out=labf[:, rs], in0=labm[:, rs], in1=bins[:, rs, 1],
                op=mybir.AluOpType.add,
            )

        # ---- PE path helpers -------------------------------------------
        C8n = work_p.tile([P, 8, D], fp8)
        labi = work_p.tile([P, R - X, 1], i32)

        def emit_c8(h):
            nc.scalar.copy(
                out=C8n[:, 4 * h:4 * h + 4, :], in_=C16n[:, 4 * h:4 * h + 4, :]
            )

        def emit_labi_half(h):
            lo = max(X, 16 * h)
            hi = 16 * (h + 1)
            nc.vector.tensor_copy(
                out=labi[:, lo - X:hi - X, 0], in_=labf[:, lo:hi]
            )

        def emit_psB(g):
            ps = psB_p.tile([P, 4, P], f32, tag="psB")
            for t in range(4):
                r = 4 * g + t
                nc.tensor.transpose(
                    out=ps[:, t, :],
                    in_=labf[:, r].to_broadcast([P, P]),
                    identity=idf32[:],
                )
            return ps

        def emit_sel(g, ps):
            labT4 = sqp.tile([P, 4, P], fp16, tag="labT4")
            nc.vector.tensor_copy(out=labT4[:], in_=ps[:])
            sel = selp.tile([P, 8, 4, P], fp8, tag="sel")
            nc.vector.tensor_tensor(
                out=sel[:],
                in0=labT4[:].unsqueeze(1).broadcast_to([P, 8, 4, P]),
                in1=ioff[:],
                op=EQ,
            )
            return sel

        pair_ps = {}

        def emit_mms_pe_pair(t0, sel):
            pr = t0 // 2
            pstile = ps2_p.tile([P, 2, D], f32, tag="ps2")
            pair_ps[pr] = pstile
            for tt in (t0, t0 + 1):
                o = pstile[:, tt % 2, :]
                for q in range(4):
                    nc.tensor.matmul(
                        out=o,
                        lhsT=sel[:, 2 * q:2 * q + 2, tt % 4, :],
                        rhs=C8n[:, 2 * q:2 * q + 2, :],
                        start=(q == 0), stop=False,
                        perf_mode=DRmode,
                    )
                nc.tensor.matmul(
                    out=o, lhsT=idf32[:, :], rhs=F32[:, tt, :],
                    start=False, stop=(tt == t0 + 1) or True,
                )

        def emit_mms_dma_pair(t0, gtile):
            pr = t0 // 2
            pstile = ps2_p.tile([P, 2, D], f32, tag="ps2")
            pair_ps[pr] = pstile
            for tt in (t0, t0 + 1):
                o = pstile[:, tt % 2, :]
                nc.tensor.matmul(
                    out=o, lhsT=idf32[:, :], rhs=F32[:, tt, :],
                    start=True, stop=False,
                )
                nc.tensor.matmul(
                    out=o, lhsT=idf16[:, :], rhs=gtile[:, tt % 4, :],
                    start=False, stop=True,
                )

        accPE = work_p.tile([P, NPAIR], f32)

        def emit_sq(pr):
            ps = pair_ps.pop(pr)
            if DBG and pr == 0:
                dcp = work_p.tile([P, 2, D], f32)
                nc.vector.tensor_copy(out=dcp[:], in_=ps[:])
                nc.sync.dma_start(d_ps[:, :, :], dcp[:])
            scr = sqp.tile([P, 2, D], fp16, tag="sqpe")
            nc.scalar.activation(
                out=scr[:], in_=ps[:],
                func=mybir.ActivationFunctionType.Square,
                accum_out=accPE[:, pr:pr + 1],
            )

        # ---- DMA path: plain single-row gathers ------------------------
        gts = {}

        def emit_gather(h):
            gt = difp.tile([P, 4, D], fp16, tag="g")
            for t in range(4):
                nc.gpsimd.indirect_dma_start(
                    out=gt[:, t, :], out_offset=None, in_=centn[:, :],
                    in_offset=bass.IndirectOffsetOnAxis(
                        ap=labi[:, 4 * h + t, :], axis=0
                    ),
                )
            gts[h] = gt
            if DBG and h == 0:
                nc.sync.dma_start(d_g[:, :, :], gt[:])

        # ---- emission schedule -----------------------------------------
        bin_half(0)
        emit_labi_half(0)
        emit_c8(0)
        bin_half(1)
        emit_labi_half(1)
        emit_c8(1)
        if DBG:
            nc.sync.dma_start(d_labf[:, :], labf[:])
            nc.sync.dma_start(d_labi[:, :], labi[:, :, 0])

        for h in range(NG_DMA):
            emit_gather(h)
        sels = {}
        psBs = {}
        psB0 = emit_psB(0)
        psBs[0] = psB0
        sel0 = emit_sel(0, psBs[0])
        sels[0] = sel0
        if DBG:
            nc.sync.dma_start(d_sel[:, :, :, :], sel0[:])
        # PE-path groups, weaving gather emissions early on the gpsimd queue
        for g in range(NG_PE):
            if g + 1 < NG_PE:
                psBn = emit_psB(g + 1)
                psBs[g + 1] = psBn
                seln = emit_sel(g + 1, psBn)
                sels[g + 1] = seln
            sel = sels.pop(g)
            for t0 in (4 * g, 4 * g + 2):
                emit_mms_pe_pair(t0, sel)
                emit_sq(t0 // 2)
        # DMA-path chunks
        for h in range(NG_DMA):
            gt = gts[h]
            for t0 in (X + 4 * h, X + 4 * h + 2):
                emit_mms_dma_pair(t0, gt)
                emit_sq(t0 // 2)
            gts.pop(h)

        # ---- final reduction -------------------------------------------
        s = work_p.tile([P, 1], f32)
        nc.vector.tensor_reduce(
            out=s[:], in_=accPE[:],
            axis=mybir.AxisListType.X, op=mybir.AluOpType.add,
        )
        psf = psS_p.tile([1, 1], f32, tag="fin")
        nc.tensor.matmul(out=psf[:], lhsT=ones1[:], rhs=s[:], start=True, stop=True)
        res = work_p.tile([1, 1], f32)
        nc.vector.tensor_copy(out=res[:], in_=psf[:])
        nc.sync.dma_start(out[:, :], res[:])

    nc.compile()
    return nc


def _consts():
    edges = np.tile(np.array(EDGES, dtype=np.float32), (P, 1))
    iofull = np.broadcast_to(
        (np.arange(P)[:, None, None, None] + 128.0 * np.arange(8)[None, :, None, None]),
        (P, 8, 4, P),
    ).astype(np.float16)
    identf = np.eye(P, dtype=np.float32)
    identh = np.eye(P, dtype=np.float16)
    ones1 = np.ones((P, 1), dtype=np.float32)
    return dict(edges=edges, iofull=np.ascontiguousarray(iofull), identf=identf,
                identh=identh, ones1=ones1)


def _get_nc():
    if "nc" not in _CACHE:
        _CACHE["nc"] = build_bass()
    return _CACHE["nc"]


def kernel(features, target, centers):
    from concourse.bass_utils import run_bass_kernel_spmd

    features = np.ascontiguousarray(features, dtype=np.float32)
    target = np.ascontiguousarray(target, dtype=np.float32)
    centneg16 = np.ascontiguousarray((-centers).astype(np.float16))
    consts = _consts()

    nc = _get_nc()
    in_maps = []
    for c in range(NCORES):
        sl = slice(c * SHARD, (c + 1) * SHARD)
        in_maps.append(
            {
                "features": np.ascontiguousarray(features[sl]),
                "target": np.ascontiguousarray(target[sl]),
                "centneg16": centneg16,
                **consts,
            }
        )
    r = run_bass_kernel_spmd(
        nc,
        in_maps,
        core_ids=list(range(NCORES)),
        trace=_CACHE.get("trace", False),
        tmpdir=_CACHE.get("tmpdir"),
    )
    _CACHE["last_results"] = r
    total = sum(float(res["out"][0, 0]) for res in r.results)
    return np.float32(total / N)

